# revision 3
# baseline (speedup 1.0000x reference)
import sys
sys.path.insert(0, '/opt/trn_rl_repo')
import numpy as np
import concourse.bass as bass
import concourse.bacc as bacc
import concourse.tile as tile
import concourse.mybir as mybir
from concourse.bass import broadcast_tensor_aps
from concourse.bass_utils import run_bass_kernel_spmd

F32 = mybir.dt.float32
AF = mybir.ActivationFunctionType
ALU = mybir.AluOpType

B, T, S = 32, 64, 128
H, A, E = 512, 256, 256
V = 32000            # padded vocab (true V-1 = 31999)
NC = 8
BL = B // NC         # 4 batch rows per core
VL = V // NC         # 4000 vocab cols per core

_cache = {}


def _badd(nc, out, a, b):
    a2, b2 = broadcast_tensor_aps(a, b)
    nc.vector.tensor_add(out, a2, b2)


def _build_A():
    nc = bacc.Bacc("TRN2", target_bir_lowering=False, debug=False,
                   enable_asserts=False, num_devices=NC)
    dt = F32
    inp = {}
    for name, shp in [
        ("encT",   [128, 4, BL, S]),      # enc^T  [d128, dt, b, s]
        ("embT",   [128, 2, T, BL]),      # emb^T  [e128, et, t, b]
        ("WhhT",   [128, 4, 3 * H]),      # W_hh^T [k128, kt, 3H]
        ("WiheT",  [128, 2, 3 * H]),      # W_ih[:, :E]^T
        ("WihcT",  [128, 4, 3 * H]),      # W_ih[:, E:]^T
        ("Wdec",   [128, 4, A]),          # W_dec  [k128, kt, A]
        ("Wenc",   [128, 4, A]),          # W_enc  [d128, dt, A]
        ("vv",     [128, 2]),             # v      [a128, at]
        ("hT0",    [128, 4, BL]),         # h0^T
        ("hbm0",   [BL, H]),              # h0 batch-major
        ("id4",    [BL, BL]),
        ("id1",    [1, 1]),
    ]:
        inp[name] = nc.dram_tensor(name, shp, dt, kind="ExternalInput")
    h_hist = nc.dram_tensor("h_hist", [128, 4, BL, T], dt, kind="ExternalOutput")
    aw_out = nc.dram_tensor("aw_out", [BL, T, S], dt, kind="ExternalOutput")

    with tile.TileContext(nc) as tc:
        with tc.tile_pool(name="const", bufs=1) as cp, \
             tc.tile_pool(name="work", bufs=2) as wp, \
             tc.tile_pool(name="state", bufs=1) as st, \
             tc.tile_pool(name="ps", bufs=1, space="PSUM") as pp:

            # ---- load constants ----
            def ld(name, shp):
                tt = cp.tile(shp, dt, tag=name)
                nc.sync.dma_start(tt[:], inp[name][:])
                return tt
            encT = ld("encT", [128, 4, BL, S])
            embT = ld("embT", [128, 2, T, BL])
            WhhT = ld("WhhT", [128, 4, 3 * H])
            WiheT = ld("WiheT", [128, 2, 3 * H])
            WihcT = ld("WihcT", [128, 4, 3 * H])
            Wdec = ld("Wdec", [128, 4, A])
            Wenc = ld("Wenc", [128, 4, A])
            vv = ld("vv", [128, 2])
            id4 = ld("id4", [BL, BL])
            id1 = ld("id1", [1, 1])

            hT = st.tile([128, 4, BL], dt, tag="hT")
            hbm = st.tile([BL, H], dt, tag="hbm")
            nc.sync.dma_start(hT[:], inp["hT0"][:])
            nc.sync.dma_start(hbm[:], inp["hbm0"][:])

            # ---- precompute enc_projA [a128, at2, b, s] ----
            epA = cp.tile([128, 2, BL, S], dt, tag="epA")
            for at in range(2):
                ps = pp.tile([128, BL, S], dt, tag="g0")
                for kt in range(4):
                    nc.tensor.matmul(ps[:], Wenc[:, kt, at * 128:(at + 1) * 128],
                                     encT[:, kt, :, :],
                                     start=(kt == 0), stop=(kt == 3))
                nc.vector.tensor_copy(epA[:, at], ps[:])

            # ---- precompute encW_b = enc_b @ Wihc^T : [s128, b, nb3, 512] ----
            encW = cp.tile([128, BL, 3, 512], dt, tag="encW")
            for b in range(BL):
                for nb in range(3):
                    ps = pp.tile([128, 512], dt, tag="g0")
                    for kt in range(4):
                        nc.tensor.matmul(ps[:], encT[:, kt, b, :],
                                         WihcT[:, kt, nb * 512:(nb + 1) * 512],
                                         start=(kt == 0), stop=(kt == 3))
                    nc.vector.tensor_copy(encW[:, b, nb], ps[:])

            ediag = st.tile([128, BL, BL], dt, tag="ediag")
            nc.vector.memset(ediag[:], 0.0)

            # ---- time loop ----
            for t in range(T):
                # dec^T [a128, at2, b]
                dps = pp.tile([128, 2, BL], dt, tag="dps")
                for at in range(2):
                    for kt in range(4):
                        nc.tensor.matmul(dps[:, at], Wdec[:, kt, at * 128:(at + 1) * 128],
                                         hT[:, kt, :], start=(kt == 0), stop=(kt == 3))
                # tanh(epA + dec)
                tres = wp.tile([128, 2, BL, S], dt, tag="tres")
                tin = wp.tile([128, 2, BL, S], dt, tag="tin")
                _badd(nc, tin[:], epA[:], dps[:, :, :, None])
                nc.scalar.activation(tres[:], tin[:], AF.Tanh)
                # energy [1, b, s]
                eps = pp.tile([1, BL, S], dt, tag="eps")
                for at in range(2):
                    nc.tensor.matmul(eps[:], vv[:, at:at + 1], tres[:, at],
                                     start=(at == 0), stop=(at == 1))
                # softmax (no mask: encoder_mask is all-ones)
                esb = wp.tile([1, BL, S], dt, tag="esb")
                nc.scalar.activation(esb[:], eps[:], AF.Exp)
                zsb = wp.tile([1, BL], dt, tag="zsb")
                nc.vector.tensor_reduce(zsb[:], esb[:], mybir.AxisListType.X, ALU.add)
                zr = wp.tile([1, BL], dt, tag="zr")
                nc.vector.reciprocal(zr[:], zsb[:])
                awsb = wp.tile([1, BL, S], dt, tag="awsb")
                a2, b2 = broadcast_tensor_aps(esb[:], zr[:, :, None])
                nc.vector.tensor_mul(awsb[:], a2, b2)
                nc.sync.dma_start(aw_out[:, t, :], awsb[:])
                # aw^T into ediag diag slots
                awT = pp.tile([128, BL], dt, tag="awT")
                for b in range(BL):
                    nc.tensor.transpose(awT[:, b:b + 1], awsb[:, b, :], id1[:])
                for b in range(BL):
                    nc.vector.tensor_copy(ediag[:, b, b:b + 1], awT[:, b:b + 1])

                # gates: r(0), z(1), xn(2 no hh), hn(3 only hh)
                gps = []
                for g in range(3):
                    ps = pp.tile([128, 512], dt, tag=f"g{g}")
                    gps.append(ps)
                    first = True
                    sl = slice(g * 512, (g + 1) * 512)
                    if g < 2:
                        for kt in range(4):
                            nc.tensor.matmul(ps[0:BL], hT[:, kt, :], WhhT[:, kt, sl],
                                             start=first, stop=False); first = False
                    for et in range(2):
                        nc.tensor.matmul(ps[0:BL], embT[:, et, t, :], WiheT[:, et, sl],
                                         start=first, stop=False); first = False
                    for kt in range(4):
                        nc.tensor.matmul(ps[0:BL], ediag[:, kt, :], encW[:, kt, g],
                                         start=first, stop=(kt == 3))
                        first = False
                hn = pp.tile([128, 512], dt, tag="hn")
                for kt in range(4):
                    nc.tensor.matmul(hn[0:BL], hT[:, kt, :], WhhT[:, kt, 1024:1536],
                                     start=(kt == 0), stop=(kt == 3))

                # gate math, batch-major [BL, 512]
                tr = wp.tile([BL, 512], dt, tag="tr")
                nc.scalar.activation(tr[:], gps[0][0:BL], AF.Tanh, scale=0.5)
                tz = wp.tile([BL, 512], dt, tag="tz")
                nc.scalar.activation(tz[:], gps[1][0:BL], AF.Tanh, scale=0.5)
                rhn = wp.tile([BL, 512], dt, tag="rhn")
                junk = wp.tile([BL, 1], dt, tag="junk")
                nc.vector.affine_mul_reduce(rhn[:], junk[:], tr[:], hn[0:BL],
                                            scale=0.5, bias=0.5)
                npre = wp.tile([BL, 512], dt, tag="npre")
                nc.vector.tensor_add(npre[:], gps[2][0:BL], rhn[:])
                ngate = wp.tile([BL, 512], dt, tag="ngate")
                nc.scalar.activation(ngate[:], npre[:], AF.Tanh)
                dd = wp.tile([BL, 512], dt, tag="dd")
                nc.vector.tensor_sub(dd[:], hbm[:], ngate[:])
                ss = wp.tile([BL, 512], dt, tag="ss")
                nc.vector.tensor_add(ss[:], hbm[:], ngate[:])
                td = wp.tile([BL, 512], dt, tag="td")
                nc.vector.tensor_mul(td[:], tz[:], dd[:])
                uu = wp.tile([BL, 512], dt, tag="uu")
                nc.vector.tensor_add(uu[:], ss[:], td[:])
                nc.vector.tensor_scalar_mul(hbm[:], uu[:], 0.5)

                # h^T update via PE transposes
                tp = pp.tile([128, 4, BL], dt, tag="tp")
                for c in range(4):
                    nc.tensor.transpose(tp[:, c, :], hbm[:, c * 128:(c + 1) * 128], id4[:])
                nc.vector.tensor_copy(hT[:], tp[:])
                nc.sync.dma_start(h_hist[:, :, :, t], hT[:])
    nc.compile()
    return nc


def _build_B():
    nc = bacc.Bacc("TRN2", target_bir_lowering=False, debug=False,
                   enable_asserts=False, num_devices=NC)
    dt = F32
    h_all = nc.dram_tensor("h_all", [128, 4, B * T], dt, kind="ExternalInput")
    Wfc = nc.dram_tensor("Wfc", [128, 4, VL], dt, kind="ExternalInput")
    out = nc.dram_tensor("out", [B * T, VL], dt, kind="ExternalOutput")
    NTS = [512] * 7 + [VL - 7 * 512]
    with tile.TileContext(nc) as tc:
        with tc.tile_pool(name="const", bufs=1) as cp, \
             tc.tile_pool(name="stage", bufs=4) as sp, \
             tc.tile_pool(name="ps", bufs=4, space="PSUM") as pp:
            hsb = cp.tile([128, 4, B * T], dt, tag="h")
            wsb = cp.tile([128, 4, VL], dt, tag="w")
            nc.sync.dma_start(hsb[:], h_all[:])
            nc.sync.dma_start(wsb[:], Wfc[:])
            for bt in range(16):
                msl = slice(bt * 128, (bt + 1) * 128)
                noff = 0
                for nb, nsz in enumerate(NTS):
                    ps = pp.tile([128, 512], dt, tag="acc")
                    for kt in range(4):
                        nc.tensor.matmul(ps[:, :nsz], hsb[:, kt, msl],
                                         wsb[:, kt, noff:noff + nsz],
                                         start=(kt == 0), stop=(kt == 3))
                    ot = sp.tile([128, 512], dt, tag="ot")
                    nc.vector.tensor_copy(ot[:, :nsz], ps[:, :nsz])
                    nc.sync.dma_start(out[msl, noff:noff + nsz], ot[:, :nsz])
                    noff += nsz
    nc.compile()
    return nc


def kernel(**inputs):
    y = np.asarray(inputs["y_decoder_input"])
    enc = np.asarray(inputs["encoder_outputs"], np.float32)
    h0 = np.asarray(inputs["decoder_init"], np.float32)
    emb = np.asarray(inputs["embedding"], np.float32)
    Wenc = np.asarray(inputs["W_enc"], np.float32)
    Wdec = np.asarray(inputs["W_dec"], np.float32)
    v = np.asarray(inputs["v"], np.float32)
    Wih = np.asarray(inputs["W_ih"], np.float32)
    Whh = np.asarray(inputs["W_hh"], np.float32)
    b_ih = np.asarray(inputs["b_ih"], np.float32)
    b_hh = np.asarray(inputs["b_hh"], np.float32)
    Wfc = np.asarray(inputs["W_fc"], np.float32)
    b_fc = np.asarray(inputs["b_fc"], np.float32)
    assert np.all(inputs["encoder_mask"] == 1) and not b_ih.any() and not b_hh.any()

    if "A" not in _cache:
        _cache["A"] = _build_A()
    if "B" not in _cache:
        _cache["B"] = _build_B()

    WhhT = np.ascontiguousarray(Whh.T).reshape(4, 128, 3 * H).transpose(1, 0, 2).copy()
    WiheT = np.ascontiguousarray(Wih[:, :E].T).reshape(2, 128, 3 * H).transpose(1, 0, 2).copy()
    WihcT = np.ascontiguousarray(Wih[:, E:].T).reshape(4, 128, 3 * H).transpose(1, 0, 2).copy()
    Wdec_r = Wdec.reshape(4, 128, A).transpose(1, 0, 2).copy()
    Wenc_r = Wenc.reshape(4, 128, A).transpose(1, 0, 2).copy()
    v_r = v.reshape(2, 128).T.copy()
    id4 = np.eye(BL, dtype=np.float32)
    id1 = np.eye(1, dtype=np.float32)

    in_maps = []
    for c in range(NC):
        bs = slice(c * BL, (c + 1) * BL)
        encc = enc[bs]                                    # [BL, S, 512]
        encT = np.ascontiguousarray(encc.transpose(2, 0, 1)) \
                 .reshape(4, 128, BL, S).transpose(1, 0, 2, 3).copy()
        embc = emb[y[bs]]                                 # [BL, T, E]
        embT = np.ascontiguousarray(embc.transpose(2, 1, 0)) \
                 .reshape(2, 128, T, BL).transpose(1, 0, 2, 3).copy()
        hbm0 = np.ascontiguousarray(h0[0, bs])            # [BL, H]
        hT0 = np.ascontiguousarray(hbm0.T).reshape(4, 128, BL).transpose(1, 0, 2).copy()
        in_maps.append(dict(encT=encT, embT=embT, WhhT=WhhT, WiheT=WiheT,
                            WihcT=WihcT, Wdec=Wdec_r, Wenc=Wenc_r, vv=v_r,
                            hT0=hT0, hbm0=hbm0, id4=id4, id1=id1))
    resA = run_bass_kernel_spmd(_cache["A"], in_maps, core_ids=list(range(NC)))

    h_all = np.empty((128, 4, B, T), np.float32)
    aw = np.empty((B, T, S), np.float32)
    for c in range(NC):
        bs = slice(c * BL, (c + 1) * BL)
        h_all[:, :, bs, :] = resA.results[c]["h_hist"]
        aw[bs] = resA.results[c]["aw_out"]
    h_all = np.ascontiguousarray(h_all.reshape(128, 4, B * T))

    Wfc_pad = np.zeros((H, V), np.float32)
    Wfc_pad[:, :V - 1] = Wfc
    in_mapsB = []
    for c in range(NC):
        wc = Wfc_pad[:, c * VL:(c + 1) * VL].reshape(4, 128, VL).transpose(1, 0, 2).copy()
        in_mapsB.append(dict(h_all=h_all, Wfc=wc))
    resB = run_bass_kernel_spmd(_cache["B"], in_mapsB, core_ids=list(range(NC)))

    outs = np.concatenate([resB.results[c]["out"] for c in range(NC)], axis=1)
    outs = outs[:, :V - 1]
    if b_fc.any():
        outs = outs + b_fc
    outputs = outs.reshape(B, T, V - 1)
    return outputs, aw


# revision 5
# speedup vs baseline: 1.2473x; 1.2473x over previous
import sys
sys.path.insert(0, '/opt/trn_rl_repo')
import numpy as np
import concourse.bass as bass
import concourse.bacc as bacc
import concourse.tile as tile
import concourse.mybir as mybir
from concourse.bass import broadcast_tensor_aps
from concourse.bass_utils import run_bass_kernel_spmd
import jax
from jax.sharding import Mesh, PartitionSpec
from jax.experimental.shard_map import shard_map
from concourse import bass2jax as b2j


class _Runner:
    """Cached PJRT executor for one Bass program (compile once, run many)."""

    def __init__(self, nc):
        b2j.install_neuronx_cc_hook()
        self.nc = nc
        pn = nc.partition_id_tensor.name if nc.partition_id_tensor else None
        self.pn = pn
        in_names, out_names, out_avals, zero_outs = [], [], [], []
        for alloc in nc.m.functions[0].allocations:
            if not isinstance(alloc, mybir.MemoryLocationSet):
                continue
            name = alloc.memorylocations[0].name
            if alloc.kind == "ExternalInput":
                if name != pn:
                    in_names.append(name)
            elif alloc.kind == "ExternalOutput":
                shape = tuple(alloc.tensor_shape)
                dtype = mybir.dt.np(alloc.dtype)
                out_names.append(name)
                out_avals.append(jax.core.ShapedArray(shape, dtype))
                zero_outs.append(np.zeros(shape, dtype))
        self.n_params = len(in_names)
        self.out_names, self.out_avals, self.zero_outs = out_names, out_avals, zero_outs
        all_names = list(in_names) + list(out_names)
        if pn is not None:
            all_names.append(pn)
        self.in_names = in_names

        def _body(*args):
            operands = list(args)
            if pn is not None:
                operands.append(b2j.partition_id_tensor())
            return tuple(_BASS_EXEC(
                *operands, out_avals=tuple(out_avals), in_names=tuple(all_names),
                out_names=tuple(out_names), lowering_input_output_aliases=(),
                sim_require_finite=True, sim_require_nnan=True, nc=nc))

        devices = jax.devices()[:NC]
        mesh = Mesh(np.asarray(devices), ("core",))
        nio = self.n_params + len(out_names)
        self.fn = jax.jit(
            shard_map(_body, mesh=mesh, in_specs=(PartitionSpec("core"),) * nio,
                      out_specs=(PartitionSpec("core"),) * len(out_names),
                      check_rep=False),
            donate_argnums=tuple(range(self.n_params, nio)), keep_unused=True)

    def __call__(self, in_maps):
        concat_in = [np.concatenate([np.asarray(m[n]) for m in in_maps], axis=0)
                     for n in self.in_names]
        concat_zeros = [np.zeros((NC * z.shape[0], *z.shape[1:]), z.dtype)
                        for z in self.zero_outs]
        outs = self.fn(*concat_in, *concat_zeros)
        return [{n: np.asarray(outs[i]).reshape(NC, *self.out_avals[i].shape)[c]
                 for i, n in enumerate(self.out_names)} for c in range(NC)]


def _bass_exec_bind(*a, **k):
    return b2j._bass_exec_p.bind(*a, **k)


_BASS_EXEC = _bass_exec_bind

F32 = mybir.dt.float32
AF = mybir.ActivationFunctionType
ALU = mybir.AluOpType

B, T, S = 32, 64, 128
H, A, E = 512, 256, 256
V = 32000            # padded vocab (true V-1 = 31999)
NC = 8
BL = B // NC         # 4 batch rows per core
VL = V // NC         # 4000 vocab cols per core

_cache = {}


def _badd(nc, out, a, b):
    a2, b2 = broadcast_tensor_aps(a, b)
    nc.vector.tensor_add(out, a2, b2)


def _build_A():
    nc = bacc.Bacc("TRN2", target_bir_lowering=False, debug=False,
                   enable_asserts=False, num_devices=NC)
    dt = F32
    inp = {}
    for name, shp in [
        ("encT",   [128, 4, BL, S]),      # enc^T  [d128, dt, b, s]
        ("embT",   [128, 2, T, BL]),      # emb^T  [e128, et, t, b]
        ("WhhT",   [128, 4, 3 * H]),      # W_hh^T [k128, kt, 3H]
        ("WiheT",  [128, 2, 3 * H]),      # W_ih[:, :E]^T
        ("WihcT",  [128, 4, 3 * H]),      # W_ih[:, E:]^T
        ("Wdec",   [128, 4, A]),          # W_dec  [k128, kt, A]
        ("Wenc",   [128, 4, A]),          # W_enc  [d128, dt, A]
        ("vv",     [128, 2]),             # v      [a128, at]
        ("hT0",    [128, 4, BL]),         # h0^T
        ("hbm0",   [BL, H]),              # h0 batch-major
        ("id4",    [BL, BL]),
        ("id1",    [1, 1]),
    ]:
        inp[name] = nc.dram_tensor(name, shp, dt, kind="ExternalInput")
    h_hist = nc.dram_tensor("h_hist", [128, 4, BL, T], dt, kind="ExternalOutput")
    aw_out = nc.dram_tensor("aw_out", [BL, T, S], dt, kind="ExternalOutput")

    with tile.TileContext(nc) as tc:
        with tc.tile_pool(name="const", bufs=1) as cp, \
             tc.tile_pool(name="work", bufs=2) as wp, \
             tc.tile_pool(name="state", bufs=1) as st, \
             tc.tile_pool(name="ps", bufs=1, space="PSUM") as pp:

            # ---- load constants ----
            def ld(name, shp):
                tt = cp.tile(shp, dt, tag=name)
                nc.sync.dma_start(tt[:], inp[name][:])
                return tt
            encT = ld("encT", [128, 4, BL, S])
            embT = ld("embT", [128, 2, T, BL])
            WhhT = ld("WhhT", [128, 4, 3 * H])
            WiheT = ld("WiheT", [128, 2, 3 * H])
            WihcT = ld("WihcT", [128, 4, 3 * H])
            Wdec = ld("Wdec", [128, 4, A])
            Wenc = ld("Wenc", [128, 4, A])
            vv = ld("vv", [128, 2])
            id4 = ld("id4", [BL, BL])
            id1 = ld("id1", [1, 1])

            hT = st.tile([128, 4, BL], dt, tag="hT")
            hbm = st.tile([BL, H], dt, tag="hbm")
            nc.sync.dma_start(hT[:], inp["hT0"][:])
            nc.sync.dma_start(hbm[:], inp["hbm0"][:])

            # ---- precompute enc_projA [a128, at2, b, s] ----
            epA = cp.tile([128, 2, BL, S], dt, tag="epA")
            for at in range(2):
                ps = pp.tile([128, BL, S], dt, tag="g0")
                for kt in range(4):
                    nc.tensor.matmul(ps[:], Wenc[:, kt, at * 128:(at + 1) * 128],
                                     encT[:, kt, :, :],
                                     start=(kt == 0), stop=(kt == 3))
                nc.vector.tensor_copy(epA[:, at], ps[:])

            # ---- precompute encW_b = enc_b @ Wihc^T : [s128, b, nb3, 512] ----
            encW = cp.tile([128, BL, 3, 512], dt, tag="encW")
            for b in range(BL):
                for nb in range(3):
                    ps = pp.tile([128, 512], dt, tag="g0")
                    for kt in range(4):
                        nc.tensor.matmul(ps[:], encT[:, kt, b, :],
                                         WihcT[:, kt, nb * 512:(nb + 1) * 512],
                                         start=(kt == 0), stop=(kt == 3))
                    nc.vector.tensor_copy(encW[:, b, nb], ps[:])

            ediag = st.tile([128, BL, BL], dt, tag="ediag")
            nc.vector.memset(ediag[:], 0.0)

            # ---- time loop ----
            for t in range(T):
                # dec^T [a128, at2, b]
                dps = pp.tile([128, 2, BL], dt, tag="dps")
                for at in range(2):
                    for kt in range(4):
                        nc.tensor.matmul(dps[:, at], Wdec[:, kt, at * 128:(at + 1) * 128],
                                         hT[:, kt, :], start=(kt == 0), stop=(kt == 3))
                # tanh(epA + dec)
                tres = wp.tile([128, 2, BL, S], dt, tag="tres")
                tin = wp.tile([128, 2, BL, S], dt, tag="tin")
                _badd(nc, tin[:], epA[:], dps[:, :, :, None])
                nc.scalar.activation(tres[:], tin[:], AF.Tanh)
                # energy [1, b, s]
                eps = pp.tile([1, BL, S], dt, tag="eps")
                for at in range(2):
                    nc.tensor.matmul(eps[:], vv[:, at:at + 1], tres[:, at],
                                     start=(at == 0), stop=(at == 1))
                # softmax (no mask: encoder_mask is all-ones)
                esb = wp.tile([1, BL, S], dt, tag="esb")
                nc.scalar.activation(esb[:], eps[:], AF.Exp)
                zsb = wp.tile([1, BL], dt, tag="zsb")
                nc.vector.tensor_reduce(zsb[:], esb[:], mybir.AxisListType.X, ALU.add)
                zr = wp.tile([1, BL], dt, tag="zr")
                nc.vector.reciprocal(zr[:], zsb[:])
                awsb = wp.tile([1, BL, S], dt, tag="awsb")
                a2, b2 = broadcast_tensor_aps(esb[:], zr[:, :, None])
                nc.vector.tensor_mul(awsb[:], a2, b2)
                nc.sync.dma_start(aw_out[:, t, :], awsb[:])
                # aw^T into ediag diag slots
                awT = pp.tile([128, BL], dt, tag="awT")
                for b in range(BL):
                    nc.tensor.transpose(awT[:, b:b + 1], awsb[:, b, :], id1[:])
                for b in range(BL):
                    nc.vector.tensor_copy(ediag[:, b, b:b + 1], awT[:, b:b + 1])

                # gates: r(0), z(1), xn(2 no hh), hn(3 only hh)
                gps = []
                for g in range(3):
                    ps = pp.tile([128, 512], dt, tag=f"g{g}")
                    gps.append(ps)
                    first = True
                    sl = slice(g * 512, (g + 1) * 512)
                    if g < 2:
                        for kt in range(4):
                            nc.tensor.matmul(ps[0:BL], hT[:, kt, :], WhhT[:, kt, sl],
                                             start=first, stop=False); first = False
                    for et in range(2):
                        nc.tensor.matmul(ps[0:BL], embT[:, et, t, :], WiheT[:, et, sl],
                                         start=first, stop=False); first = False
                    for kt in range(4):
                        nc.tensor.matmul(ps[0:BL], ediag[:, kt, :], encW[:, kt, g],
                                         start=first, stop=(kt == 3))
                        first = False
                hn = pp.tile([128, 512], dt, tag="hn")
                for kt in range(4):
                    nc.tensor.matmul(hn[0:BL], hT[:, kt, :], WhhT[:, kt, 1024:1536],
                                     start=(kt == 0), stop=(kt == 3))

                # gate math, batch-major [BL, 512]
                tr = wp.tile([BL, 512], dt, tag="tr")
                nc.scalar.activation(tr[:], gps[0][0:BL], AF.Tanh, scale=0.5)
                tz = wp.tile([BL, 512], dt, tag="tz")
                nc.scalar.activation(tz[:], gps[1][0:BL], AF.Tanh, scale=0.5)
                rhn = wp.tile([BL, 512], dt, tag="rhn")
                junk = wp.tile([BL, 1], dt, tag="junk")
                nc.vector.affine_mul_reduce(rhn[:], junk[:], tr[:], hn[0:BL],
                                            scale=0.5, bias=0.5)
                npre = wp.tile([BL, 512], dt, tag="npre")
                nc.vector.tensor_add(npre[:], gps[2][0:BL], rhn[:])
                ngate = wp.tile([BL, 512], dt, tag="ngate")
                nc.scalar.activation(ngate[:], npre[:], AF.Tanh)
                dd = wp.tile([BL, 512], dt, tag="dd")
                nc.vector.tensor_sub(dd[:], hbm[:], ngate[:])
                ss = wp.tile([BL, 512], dt, tag="ss")
                nc.vector.tensor_add(ss[:], hbm[:], ngate[:])
                td = wp.tile([BL, 512], dt, tag="td")
                nc.vector.tensor_mul(td[:], tz[:], dd[:])
                uu = wp.tile([BL, 512], dt, tag="uu")
                nc.vector.tensor_add(uu[:], ss[:], td[:])
                nc.vector.tensor_scalar_mul(hbm[:], uu[:], 0.5)

                # h^T update via PE transposes
                tp = pp.tile([128, 4, BL], dt, tag="tp")
                for c in range(4):
                    nc.tensor.transpose(tp[:, c, :], hbm[:, c * 128:(c + 1) * 128], id4[:])
                nc.vector.tensor_copy(hT[:], tp[:])
                nc.sync.dma_start(h_hist[:, :, :, t], hT[:])
    nc.compile()
    return nc


def _build_B():
    nc = bacc.Bacc("TRN2", target_bir_lowering=False, debug=False,
                   enable_asserts=False, num_devices=NC)
    dt = F32
    h_all = nc.dram_tensor("h_all", [128, 4, B * T], dt, kind="ExternalInput")
    Wfc = nc.dram_tensor("Wfc", [128, 4, VL], dt, kind="ExternalInput")
    out = nc.dram_tensor("out", [B * T, VL], dt, kind="ExternalOutput")
    NTS = [512] * 7 + [VL - 7 * 512]
    with tile.TileContext(nc) as tc:
        with tc.tile_pool(name="const", bufs=1) as cp, \
             tc.tile_pool(name="stage", bufs=4) as sp, \
             tc.tile_pool(name="ps", bufs=4, space="PSUM") as pp:
            hsb = cp.tile([128, 4, B * T], dt, tag="h")
            wsb = cp.tile([128, 4, VL], dt, tag="w")
            nc.sync.dma_start(hsb[:], h_all[:])
            nc.sync.dma_start(wsb[:], Wfc[:])
            for bt in range(16):
                msl = slice(bt * 128, (bt + 1) * 128)
                noff = 0
                for nb, nsz in enumerate(NTS):
                    ps = pp.tile([128, 512], dt, tag="acc")
                    for kt in range(4):
                        nc.tensor.matmul(ps[:, :nsz], hsb[:, kt, msl],
                                         wsb[:, kt, noff:noff + nsz],
                                         start=(kt == 0), stop=(kt == 3))
                    ot = sp.tile([128, 512], dt, tag="ot")
                    nc.vector.tensor_copy(ot[:, :nsz], ps[:, :nsz])
                    nc.sync.dma_start(out[msl, noff:noff + nsz], ot[:, :nsz])
                    noff += nsz
    nc.compile()
    return nc


def kernel(**inputs):
    y = np.asarray(inputs["y_decoder_input"])
    enc = np.asarray(inputs["encoder_outputs"], np.float32)
    h0 = np.asarray(inputs["decoder_init"], np.float32)
    emb = np.asarray(inputs["embedding"], np.float32)
    Wenc = np.asarray(inputs["W_enc"], np.float32)
    Wdec = np.asarray(inputs["W_dec"], np.float32)
    v = np.asarray(inputs["v"], np.float32)
    Wih = np.asarray(inputs["W_ih"], np.float32)
    Whh = np.asarray(inputs["W_hh"], np.float32)
    b_ih = np.asarray(inputs["b_ih"], np.float32)
    b_hh = np.asarray(inputs["b_hh"], np.float32)
    Wfc = np.asarray(inputs["W_fc"], np.float32)
    b_fc = np.asarray(inputs["b_fc"], np.float32)
    assert np.all(inputs["encoder_mask"] == 1) and not b_ih.any() and not b_hh.any()

    if "A" not in _cache:
        _cache["A"] = _build_A()
    if "B" not in _cache:
        _cache["B"] = _build_B()

    WhhT = np.ascontiguousarray(Whh.T).reshape(4, 128, 3 * H).transpose(1, 0, 2).copy()
    WiheT = np.ascontiguousarray(Wih[:, :E].T).reshape(2, 128, 3 * H).transpose(1, 0, 2).copy()
    WihcT = np.ascontiguousarray(Wih[:, E:].T).reshape(4, 128, 3 * H).transpose(1, 0, 2).copy()
    Wdec_r = Wdec.reshape(4, 128, A).transpose(1, 0, 2).copy()
    Wenc_r = Wenc.reshape(4, 128, A).transpose(1, 0, 2).copy()
    v_r = v.reshape(2, 128).T.copy()
    id4 = np.eye(BL, dtype=np.float32)
    id1 = np.eye(1, dtype=np.float32)

    in_maps = []
    for c in range(NC):
        bs = slice(c * BL, (c + 1) * BL)
        encc = enc[bs]                                    # [BL, S, 512]
        encT = np.ascontiguousarray(encc.transpose(2, 0, 1)) \
                 .reshape(4, 128, BL, S).transpose(1, 0, 2, 3).copy()
        embc = emb[y[bs]]                                 # [BL, T, E]
        embT = np.ascontiguousarray(embc.transpose(2, 1, 0)) \
                 .reshape(2, 128, T, BL).transpose(1, 0, 2, 3).copy()
        hbm0 = np.ascontiguousarray(h0[0, bs])            # [BL, H]
        hT0 = np.ascontiguousarray(hbm0.T).reshape(4, 128, BL).transpose(1, 0, 2).copy()
        in_maps.append(dict(encT=encT, embT=embT, WhhT=WhhT, WiheT=WiheT,
                            WihcT=WihcT, Wdec=Wdec_r, Wenc=Wenc_r, vv=v_r,
                            hT0=hT0, hbm0=hbm0, id4=id4, id1=id1))
    if "runA" not in _cache:
        _cache["runA"] = _Runner(_cache["A"])
    resA = _cache["runA"](in_maps)

    h_all = np.empty((128, 4, B, T), np.float32)
    aw = np.empty((B, T, S), np.float32)
    for c in range(NC):
        bs = slice(c * BL, (c + 1) * BL)
        h_all[:, :, bs, :] = resA[c]["h_hist"]
        aw[bs] = resA[c]["aw_out"]
    h_all = np.ascontiguousarray(h_all.reshape(128, 4, B * T))

    Wfc_pad = np.zeros((H, V), np.float32)
    Wfc_pad[:, :V - 1] = Wfc
    in_mapsB = []
    for c in range(NC):
        wc = Wfc_pad[:, c * VL:(c + 1) * VL].reshape(4, 128, VL).transpose(1, 0, 2).copy()
        in_mapsB.append(dict(h_all=h_all, Wfc=wc))
    if "runB" not in _cache:
        _cache["runB"] = _Runner(_cache["B"])
    resB = _cache["runB"](in_mapsB)

    outs = np.concatenate([resB[c]["out"] for c in range(NC)], axis=1)
    outs = outs[:, :V - 1]
    if b_fc.any():
        outs = outs + b_fc
    outputs = outs.reshape(B, T, V - 1)
    return outputs, aw


# revision 6
# speedup vs baseline: 1.3179x; 1.0566x over previous
import sys
sys.path.insert(0, '/opt/trn_rl_repo')
import numpy as np
import concourse.bass as bass
import concourse.bacc as bacc
import concourse.tile as tile
import concourse.mybir as mybir
from concourse.bass import broadcast_tensor_aps
from concourse.bass_utils import run_bass_kernel_spmd
import jax
from jax.sharding import Mesh, PartitionSpec
from jax.experimental.shard_map import shard_map
from concourse import bass2jax as b2j


class _Runner:
    """Cached PJRT executor for one Bass program (compile once, run many)."""

    def __init__(self, nc):
        b2j.install_neuronx_cc_hook()
        self.nc = nc
        pn = nc.partition_id_tensor.name if nc.partition_id_tensor else None
        self.pn = pn
        in_names, out_names, out_avals, zero_outs = [], [], [], []
        for alloc in nc.m.functions[0].allocations:
            if not isinstance(alloc, mybir.MemoryLocationSet):
                continue
            name = alloc.memorylocations[0].name
            if alloc.kind == "ExternalInput":
                if name != pn:
                    in_names.append(name)
            elif alloc.kind == "ExternalOutput":
                shape = tuple(alloc.tensor_shape)
                dtype = mybir.dt.np(alloc.dtype)
                out_names.append(name)
                out_avals.append(jax.core.ShapedArray(shape, dtype))
                zero_outs.append(np.zeros(shape, dtype))
        self.n_params = len(in_names)
        self.out_names, self.out_avals, self.zero_outs = out_names, out_avals, zero_outs
        all_names = list(in_names) + list(out_names)
        if pn is not None:
            all_names.append(pn)
        self.in_names = in_names

        def _body(*args):
            operands = list(args)
            if pn is not None:
                operands.append(b2j.partition_id_tensor())
            return tuple(_BASS_EXEC(
                *operands, out_avals=tuple(out_avals), in_names=tuple(all_names),
                out_names=tuple(out_names), lowering_input_output_aliases=(),
                sim_require_finite=True, sim_require_nnan=True, nc=nc))

        devices = jax.devices()[:NC]
        mesh = Mesh(np.asarray(devices), ("core",))
        nio = self.n_params + len(out_names)
        self.fn = jax.jit(
            shard_map(_body, mesh=mesh, in_specs=(PartitionSpec("core"),) * nio,
                      out_specs=(PartitionSpec("core"),) * len(out_names),
                      check_rep=False),
            donate_argnums=tuple(range(self.n_params, nio)), keep_unused=True)

    def __call__(self, in_maps):
        concat_in = [np.concatenate([np.asarray(m[n]) for m in in_maps], axis=0)
                     for n in self.in_names]
        concat_zeros = [np.zeros((NC * z.shape[0], *z.shape[1:]), z.dtype)
                        for z in self.zero_outs]
        outs = self.fn(*concat_in, *concat_zeros)
        return [{n: np.asarray(outs[i]).reshape(NC, *self.out_avals[i].shape)[c]
                 for i, n in enumerate(self.out_names)} for c in range(NC)]


def _bass_exec_bind(*a, **k):
    return b2j._bass_exec_p.bind(*a, **k)


_BASS_EXEC = _bass_exec_bind

F32 = mybir.dt.float32
AF = mybir.ActivationFunctionType
ALU = mybir.AluOpType

B, T, S = 32, 64, 128
H, A, E = 512, 256, 256
V = 32000            # padded vocab (true V-1 = 31999)
NC = 8
BL = B // NC         # 4 batch rows per core
VL = V // NC         # 4000 vocab cols per core

_cache = {}


def _badd(nc, out, a, b):
    a2, b2 = broadcast_tensor_aps(a, b)
    nc.vector.tensor_add(out, a2, b2)


def _build_A():
    nc = bacc.Bacc("TRN2", target_bir_lowering=False, debug=False,
                   enable_asserts=False, num_devices=NC)
    dt = F32
    inp = {}
    for name, shp in [
        ("encT",   [128, 4, BL, S]),      # enc^T  [d128, dt, b, s]
        ("embT",   [128, 2, T, BL]),      # emb^T  [e128, et, t, b]
        ("WhhT",   [128, 4, 3 * H]),      # W_hh^T [k128, kt, 3H]
        ("WiheT",  [128, 2, 3 * H]),      # W_ih[:, :E]^T
        ("WihcT",  [128, 4, 3 * H]),      # W_ih[:, E:]^T
        ("Wdec",   [128, 4, A]),          # W_dec  [k128, kt, A]
        ("Wenc",   [128, 4, A]),          # W_enc  [d128, dt, A]
        ("vv",     [128, 2]),             # v      [a128, at]
        ("hT0",    [128, 4, BL]),         # h0^T
        ("hbm0",   [BL, H]),              # h0 batch-major
        ("id4",    [BL, BL]),
        ("id1",    [1, 1]),
    ]:
        inp[name] = nc.dram_tensor(name, shp, dt, kind="ExternalInput")
    h_hist = nc.dram_tensor("h_hist", [T, 128, 4, BL], dt, kind="ExternalOutput")
    aw_out = nc.dram_tensor("aw_out", [BL, T, S], dt, kind="ExternalOutput")

    with tile.TileContext(nc) as tc:
        with tc.tile_pool(name="const", bufs=1) as cp, \
             tc.tile_pool(name="work", bufs=2) as wp, \
             tc.tile_pool(name="state", bufs=1) as st, \
             tc.tile_pool(name="ps", bufs=1, space="PSUM") as pp:

            # ---- load constants ----
            def ld(name, shp):
                tt = cp.tile(shp, dt, tag=name)
                nc.sync.dma_start(tt[:], inp[name][:])
                return tt
            encT = ld("encT", [128, 4, BL, S])
            embT = ld("embT", [128, 2, T, BL])
            WhhT = ld("WhhT", [128, 4, 3 * H])
            WiheT = ld("WiheT", [128, 2, 3 * H])
            WihcT = ld("WihcT", [128, 4, 3 * H])
            Wdec = ld("Wdec", [128, 4, A])
            Wenc = ld("Wenc", [128, 4, A])
            vv = ld("vv", [128, 2])
            id4 = ld("id4", [BL, BL])
            id1 = ld("id1", [1, 1])

            hT = st.tile([128, 4, BL], dt, tag="hT")
            hbm = st.tile([BL, H], dt, tag="hbm")
            nc.sync.dma_start(hT[:], inp["hT0"][:])
            nc.sync.dma_start(hbm[:], inp["hbm0"][:])

            # ---- precompute enc_projA [a128, at2, b, s] ----
            epA = cp.tile([128, 2, BL, S], dt, tag="epA")
            for at in range(2):
                ps = pp.tile([128, BL, S], dt, tag="g0")
                for kt in range(4):
                    nc.tensor.matmul(ps[:], Wenc[:, kt, at * 128:(at + 1) * 128],
                                     encT[:, kt, :, :],
                                     start=(kt == 0), stop=(kt == 3))
                nc.vector.tensor_copy(epA[:, at], ps[:])

            # ---- precompute encW_b = enc_b @ Wihc^T : [s128, b, nb3, 512] ----
            encW = cp.tile([128, BL, 3, 512], dt, tag="encW")
            for b in range(BL):
                for nb in range(3):
                    ps = pp.tile([128, 512], dt, tag="g0")
                    for kt in range(4):
                        nc.tensor.matmul(ps[:], encT[:, kt, b, :],
                                         WihcT[:, kt, nb * 512:(nb + 1) * 512],
                                         start=(kt == 0), stop=(kt == 3))
                    nc.vector.tensor_copy(encW[:, b, nb], ps[:])

            ediag = st.tile([128, BL, BL], dt, tag="ediag")
            nc.vector.memset(ediag[:], 0.0)

            # ---- time loop ----
            for t in range(T):
                # dec^T [a128, at2, b]
                dps = pp.tile([128, 2, BL], dt, tag="dps")
                for at in range(2):
                    for kt in range(4):
                        nc.tensor.matmul(dps[:, at], Wdec[:, kt, at * 128:(at + 1) * 128],
                                         hT[:, kt, :], start=(kt == 0), stop=(kt == 3))
                # tanh(epA + dec)
                tres = wp.tile([128, 2, BL, S], dt, tag="tres")
                tin = wp.tile([128, 2, BL, S], dt, tag="tin")
                _badd(nc, tin[:], epA[:], dps[:, :, :, None])
                nc.scalar.activation(tres[:], tin[:], AF.Tanh)
                # energy [1, b, s]
                eps = pp.tile([1, BL, S], dt, tag="eps")
                for at in range(2):
                    nc.tensor.matmul(eps[:], vv[:, at:at + 1], tres[:, at],
                                     start=(at == 0), stop=(at == 1))
                # softmax (no mask: encoder_mask is all-ones)
                esb = wp.tile([1, BL, S], dt, tag="esb")
                nc.scalar.activation(esb[:], eps[:], AF.Exp)
                zsb = wp.tile([1, BL], dt, tag="zsb")
                nc.vector.tensor_reduce(zsb[:], esb[:], mybir.AxisListType.X, ALU.add)
                zr = wp.tile([1, BL], dt, tag="zr")
                nc.vector.reciprocal(zr[:], zsb[:])
                awsb = wp.tile([1, BL, S], dt, tag="awsb")
                a2, b2 = broadcast_tensor_aps(esb[:], zr[:, :, None])
                nc.vector.tensor_mul(awsb[:], a2, b2)
                nc.sync.dma_start(aw_out[:, t, :], awsb[:])
                # aw^T into ediag diag slots
                awT = pp.tile([128, BL], dt, tag="awT")
                for b in range(BL):
                    nc.tensor.transpose(awT[:, b:b + 1], awsb[:, b, :], id1[:])
                for b in range(BL):
                    nc.vector.tensor_copy(ediag[:, b, b:b + 1], awT[:, b:b + 1])

                # gates: r(0), z(1), xn(2 no hh), hn(3 only hh)
                gps = []
                for g in range(3):
                    ps = pp.tile([128, 512], dt, tag=f"g{g}")
                    gps.append(ps)
                    first = True
                    sl = slice(g * 512, (g + 1) * 512)
                    if g < 2:
                        for kt in range(4):
                            nc.tensor.matmul(ps[0:BL], hT[:, kt, :], WhhT[:, kt, sl],
                                             start=first, stop=False); first = False
                    for et in range(2):
                        nc.tensor.matmul(ps[0:BL], embT[:, et, t, :], WiheT[:, et, sl],
                                         start=first, stop=False); first = False
                    for kt in range(4):
                        nc.tensor.matmul(ps[0:BL], ediag[:, kt, :], encW[:, kt, g],
                                         start=first, stop=(kt == 3))
                        first = False
                hn = pp.tile([128, 512], dt, tag="hn")
                for kt in range(4):
                    nc.tensor.matmul(hn[0:BL], hT[:, kt, :], WhhT[:, kt, 1024:1536],
                                     start=(kt == 0), stop=(kt == 3))

                # gate math, batch-major [BL, 512]
                tr = wp.tile([BL, 512], dt, tag="tr")
                nc.scalar.activation(tr[:], gps[0][0:BL], AF.Tanh, scale=0.5)
                tz = wp.tile([BL, 512], dt, tag="tz")
                nc.scalar.activation(tz[:], gps[1][0:BL], AF.Tanh, scale=0.5)
                rhn = wp.tile([BL, 512], dt, tag="rhn")
                junk = wp.tile([BL, 1], dt, tag="junk")
                nc.vector.affine_mul_reduce(rhn[:], junk[:], tr[:], hn[0:BL],
                                            scale=0.5, bias=0.5)
                npre = wp.tile([BL, 512], dt, tag="npre")
                nc.vector.tensor_add(npre[:], gps[2][0:BL], rhn[:])
                ngate = wp.tile([BL, 512], dt, tag="ngate")
                nc.scalar.activation(ngate[:], npre[:], AF.Tanh)
                dd = wp.tile([BL, 512], dt, tag="dd")
                nc.vector.tensor_sub(dd[:], hbm[:], ngate[:])
                ss = wp.tile([BL, 512], dt, tag="ss")
                nc.vector.tensor_add(ss[:], hbm[:], ngate[:])
                td = wp.tile([BL, 512], dt, tag="td")
                nc.vector.tensor_mul(td[:], tz[:], dd[:])
                uu = wp.tile([BL, 512], dt, tag="uu")
                nc.vector.tensor_add(uu[:], ss[:], td[:])
                nc.vector.tensor_scalar_mul(hbm[:], uu[:], 0.5)

                # h^T update via PE transposes
                tp = pp.tile([128, 4, BL], dt, tag="tp")
                for c in range(4):
                    nc.tensor.transpose(tp[:, c, :], hbm[:, c * 128:(c + 1) * 128], id4[:])
                nc.vector.tensor_copy(hT[:], tp[:])
                nc.sync.dma_start(h_hist[t], hT[:])
    nc.compile()
    return nc


def _build_B():
    nc = bacc.Bacc("TRN2", target_bir_lowering=False, debug=False,
                   enable_asserts=False, num_devices=NC)
    dt = F32
    h_all = nc.dram_tensor("h_all", [128, 4, B * T], dt, kind="ExternalInput")
    Wfc = nc.dram_tensor("Wfc", [128, 4, VL], dt, kind="ExternalInput")
    out = nc.dram_tensor("out", [B * T, VL], dt, kind="ExternalOutput")
    NTS = [512] * 7 + [VL - 7 * 512]
    with tile.TileContext(nc) as tc:
        with tc.tile_pool(name="const", bufs=1) as cp, \
             tc.tile_pool(name="stage", bufs=4) as sp, \
             tc.tile_pool(name="ps", bufs=4, space="PSUM") as pp:
            hsb = cp.tile([128, 4, B * T], dt, tag="h")
            wsb = cp.tile([128, 4, VL], dt, tag="w")
            nc.sync.dma_start(hsb[:], h_all[:])
            nc.sync.dma_start(wsb[:], Wfc[:])
            for bt in range(16):
                msl = slice(bt * 128, (bt + 1) * 128)
                noff = 0
                for nb, nsz in enumerate(NTS):
                    ps = pp.tile([128, 512], dt, tag="acc")
                    for kt in range(4):
                        nc.tensor.matmul(ps[:, :nsz], hsb[:, kt, msl],
                                         wsb[:, kt, noff:noff + nsz],
                                         start=(kt == 0), stop=(kt == 3))
                    ot = sp.tile([128, 512], dt, tag="ot")
                    nc.vector.tensor_copy(ot[:, :nsz], ps[:, :nsz])
                    nc.sync.dma_start(out[msl, noff:noff + nsz], ot[:, :nsz])
                    noff += nsz
    nc.compile()
    return nc


def kernel(**inputs):
    y = np.asarray(inputs["y_decoder_input"])
    enc = np.asarray(inputs["encoder_outputs"], np.float32)
    h0 = np.asarray(inputs["decoder_init"], np.float32)
    emb = np.asarray(inputs["embedding"], np.float32)
    Wenc = np.asarray(inputs["W_enc"], np.float32)
    Wdec = np.asarray(inputs["W_dec"], np.float32)
    v = np.asarray(inputs["v"], np.float32)
    Wih = np.asarray(inputs["W_ih"], np.float32)
    Whh = np.asarray(inputs["W_hh"], np.float32)
    b_ih = np.asarray(inputs["b_ih"], np.float32)
    b_hh = np.asarray(inputs["b_hh"], np.float32)
    Wfc = np.asarray(inputs["W_fc"], np.float32)
    b_fc = np.asarray(inputs["b_fc"], np.float32)
    assert np.all(inputs["encoder_mask"] == 1) and not b_ih.any() and not b_hh.any()

    if "A" not in _cache:
        _cache["A"] = _build_A()
    if "B" not in _cache:
        _cache["B"] = _build_B()

    WhhT = np.ascontiguousarray(Whh.T).reshape(4, 128, 3 * H).transpose(1, 0, 2).copy()
    WiheT = np.ascontiguousarray(Wih[:, :E].T).reshape(2, 128, 3 * H).transpose(1, 0, 2).copy()
    WihcT = np.ascontiguousarray(Wih[:, E:].T).reshape(4, 128, 3 * H).transpose(1, 0, 2).copy()
    Wdec_r = Wdec.reshape(4, 128, A).transpose(1, 0, 2).copy()
    Wenc_r = Wenc.reshape(4, 128, A).transpose(1, 0, 2).copy()
    v_r = v.reshape(2, 128).T.copy()
    id4 = np.eye(BL, dtype=np.float32)
    id1 = np.eye(1, dtype=np.float32)

    in_maps = []
    for c in range(NC):
        bs = slice(c * BL, (c + 1) * BL)
        encc = enc[bs]                                    # [BL, S, 512]
        encT = np.ascontiguousarray(encc.transpose(2, 0, 1)) \
                 .reshape(4, 128, BL, S).transpose(1, 0, 2, 3).copy()
        embc = emb[y[bs]]                                 # [BL, T, E]
        embT = np.ascontiguousarray(embc.transpose(2, 1, 0)) \
                 .reshape(2, 128, T, BL).transpose(1, 0, 2, 3).copy()
        hbm0 = np.ascontiguousarray(h0[0, bs])            # [BL, H]
        hT0 = np.ascontiguousarray(hbm0.T).reshape(4, 128, BL).transpose(1, 0, 2).copy()
        in_maps.append(dict(encT=encT, embT=embT, WhhT=WhhT, WiheT=WiheT,
                            WihcT=WihcT, Wdec=Wdec_r, Wenc=Wenc_r, vv=v_r,
                            hT0=hT0, hbm0=hbm0, id4=id4, id1=id1))
    if "runA" not in _cache:
        _cache["runA"] = _Runner(_cache["A"])
    resA = _cache["runA"](in_maps)

    h_all = np.empty((128, 4, B, T), np.float32)
    aw = np.empty((B, T, S), np.float32)
    for c in range(NC):
        bs = slice(c * BL, (c + 1) * BL)
        h_all[:, :, bs, :] = resA[c]["h_hist"].transpose(1, 2, 3, 0)
        aw[bs] = resA[c]["aw_out"]
    h_all = np.ascontiguousarray(h_all.reshape(128, 4, B * T))

    Wfc_pad = np.zeros((H, V), np.float32)
    Wfc_pad[:, :V - 1] = Wfc
    in_mapsB = []
    for c in range(NC):
        wc = Wfc_pad[:, c * VL:(c + 1) * VL].reshape(4, 128, VL).transpose(1, 0, 2).copy()
        in_mapsB.append(dict(h_all=h_all, Wfc=wc))
    if "runB" not in _cache:
        _cache["runB"] = _Runner(_cache["B"])
    resB = _cache["runB"](in_mapsB)

    outs = np.concatenate([resB[c]["out"] for c in range(NC)], axis=1)
    outs = outs[:, :V - 1]
    if b_fc.any():
        outs = outs + b_fc
    outputs = outs.reshape(B, T, V - 1)
    return outputs, aw
